# revision 24
# baseline (speedup 1.0000x reference)
"""AttnBlock (GroupNorm + single-head spatial self-attention + residual) on
8 Trainium2 NeuronCores — fp8 DoubleRow edition.

Sharding: batch (4) x query-half (2) -> 8 independent shards, one per core.
The host rolls the flattened spatial axis by 2048 for odd cores so each
core's queries are the first 2048 columns of its local x; K/V see all 4096.

Host preprocessing (all per batch, standard norm/weight folding):
  - GroupNorm stats (mean/var over 32ch x 4096) -> alpha/beta; folded into
    the conv weights:  A = diag(alpha) (Wq^T Wk) diag(alpha)  (scores
    bilinear form, the M-trick: per-query affine cancels under softmax,
    per-key O(mean) term dropped),  Wsv = diag(alpha) Wv^T Wp^T (V conv
    with the output projection folded in), bpp = bp + Wp(bv + Wv beta).
  - Weights shipped as fp8e4 scaled x16 (into fp8's sweet spot); x shipped
    twice: fp8e4 (matmul operand) and bf16 queries (residual).

Device pipeline per core, all matmuls fp8 DoubleRow (contraction 256 in one
pass, 2 MACs/cell/cycle):
  1. kconv: k_sb[c, n] = fp8(16 A x)    (8 x 2 DR matmuls, DMA-paced)
  2. vconv: vT[n, c]  = fp8(16 Wsv^T x) (32 DR matmuls, x stationary)
  3. 4 query chunks of 512: scores st[j,q] = k_sb^T x8 (DR, pair tiles in
     2 PSUM banks), P = exp(st/256 - shift) -> fp8e5 pair tiles (ScalarE,
     per-partition bias carries the shift; e5m2 makes overflow impossible),
     PV: a[c,q] += vT pair^T P pair (DR), Z accumulated on the PE with a
     [128,2,1] ones DoubleRow matmul per pair — no partition-reduction on
     DVE at all.
  4. Epilogue per chunk (DVE+GpSimd): a * 1/(16Z) + (x + bpp), streamed out.
"""
import numpy as np

B, C, H, W = 4, 256, 64, 64
N = H * W            # 4096 spatial positions
NQ = N // 2          # 2048 queries per core
P = 128              # partitions
CT = C // P          # 2 channel tiles
NUM_GROUPS = 8
EPS = 1e-5
WSCALE = 16.0        # fp8 weight prescale
EXP_SCALE = 1.0 / 256.0   # score descale: 1/16 (attn) * 1/16 (WSCALE)
WARM_MMS = 3

_CACHED = {}


def _build():
    import concourse.bass as bass
    import concourse.mybir as mybir
    import concourse.tile as tile
    from concourse import bacc

    dt = mybir.dt
    AF = mybir.ActivationFunctionType
    DR = mybir.MatmulPerfMode.DoubleRow

    nc = bacc.Bacc("TRN2", debug=False, num_devices=8)

    x8_d = nc.dram_tensor("x8", [P, CT * N], dt.float8e4, kind="ExternalInput")
    xq_d = nc.dram_tensor("xq", [P, CT * NQ], dt.bfloat16, kind="ExternalInput")
    # wm = [packed 16*A | packed 16*Wsv], each [P, CT*C]
    wm_d = nc.dram_tensor("wm", [P, 2 * CT * C], dt.float8e4, kind="ExternalInput")
    aux_d = nc.dram_tensor("aux", [P, 8], dt.float32, kind="ExternalInput")
    out_d = nc.dram_tensor("out", [C, NQ], dt.float32, kind="ExternalOutput")

    x8_ap = x8_d.ap()
    xq_ap = xq_d.ap()
    out_ap = out_d.ap().rearrange("(t p) n -> p t n", p=P)

    with tile.TileContext(nc) as tc:
        with (
            nc.allow_low_precision(reason="fp8 attention is intentional"),
            tc.tile_pool(name="persist", bufs=1) as pe_,
            tc.tile_pool(name="pt", bufs=6) as ptp,
            tc.tile_pool(name="tmp", bufs=3) as tmp,
            tc.tile_pool(name="mm", bufs=2, space="PSUM") as mmp,
            tc.tile_pool(name="acc", bufs=2, space="PSUM") as accp,
            tc.tile_pool(name="zp", bufs=1, space="PSUM") as zpp,
            tc.tile_pool(name="vp", bufs=1, space="PSUM") as vpp,
        ):
            # ---------- DMAs first: queue engines must trigger before any
            # other work lands on them (first transfer has ~3.5us ramp) ----
            x8_r = pe_.tile([P, CT, N], dt.float8e4, tag="x8")
            x8_flat = x8_r.rearrange("p t n -> p (t n)")
            wm_sb = pe_.tile([P, 2, CT, C], dt.float8e4, tag="wm")
            nc.gpsimd.dma_start(wm_sb.rearrange("p s t o -> p (s t o)"), wm_d.ap())
            # progressive chunk sizes: first kconv can start ~1.5us earlier
            edges = [0, 512, 1024, 2048, 4096]
            for ckb in range(4):
                for t in range(CT):
                    fs = slice(t * N + edges[ckb], t * N + edges[ckb + 1])
                    eng = nc.sync if t == 0 else nc.scalar
                    eng.dma_start(x8_flat[:, fs], x8_ap[:, fs])
            aux_sb = pe_.tile([P, 8], dt.float32, tag="aux")
            nc.gpsimd.dma_start(aux_sb, aux_d.ap())
            xq_r = pe_.tile([P, CT, NQ], dt.bfloat16, tag="xq")
            nc.gpsimd.dma_start(
                xq_r.rearrange("p t n -> p (t n)"), xq_ap
            )

            # ---------- constants + PE warm-up ----------
            warm_w = pe_.tile([P, P], dt.bfloat16, tag="warmw")
            nc.vector.memset(warm_w, 0.0)
            warm_x = pe_.tile([P, 512], dt.bfloat16, tag="warmx")
            nc.vector.memset(warm_x, 0.0)
            # pair-dim byte stride must be %16 for DoubleRow ldweights
            ones2_t = pe_.tile([P, 2, 16], dt.float8e5, tag="ones2")
            nc.vector.memset(ones2_t.rearrange("p a b -> p (a b)"), 1.0)
            ones2 = ones2_t[:, :, 0:1]
            ones_row = pe_.tile([1, P], dt.bfloat16, tag="ones1r")
            nc.vector.memset(ones_row, 1.0)
            tjunk = pe_.tile([1, 2], dt.float32, tag="tjunk")
            nc.vector.memset(tjunk, 1.0)
            nc.scalar.activation(tjunk, tjunk, AF.Exp)
            for _ in range(WARM_MMS):
                wps = mmp.tile([P, 2, 512], dt.float32, tag="mm")
                nc.tensor.matmul(wps[:, 0], warm_w, warm_x, start=True, stop=True)
            wmk = wm_sb[:, 0]   # [P, CT, C] : 16*A packed
            wmv = wm_sb[:, 1]   # [P, CT, C] : 16*Wsv packed
            bpp = aux_sb[:, 0:2]
            ebias = aux_sb[:, 2:3]  # -shift

            # ---------- k conv (DoubleRow) ----------
            k_sb = pe_.tile([P, CT, N], dt.float8e4, tag="k")

            def eng_copy(eng, dst, src):
                if eng is nc.scalar:
                    eng.mul(dst, src, 1.0)
                else:
                    eng.tensor_copy(dst, src)

            def kconv(ck, copy_eng):
                s = slice(ck * 512, (ck + 1) * 512)
                cp2 = mmp.tile([P, 2, 512], dt.float32, tag="mm")
                for h in range(CT):
                    nc.tensor.matmul(
                        cp2[:, h],
                        wmk[:, :, h * P : (h + 1) * P],
                        x8_r[:, :, s],
                        start=True, stop=True, perf_mode=DR,
                    )
                eng_copy(copy_eng, k_sb[:, :, s], cp2)

            # ---------- v conv (DoubleRow, x stationary -> transposed out) ----------
            vT = pe_.tile([P, 32, C], dt.float8e4, tag="vT")

            def vconv(u, copy_eng):
                vp2 = vpp.tile([P, 2, C], dt.float32, tag="vp")
                for i in range(2):
                    jt = 2 * u + i
                    nc.tensor.matmul(
                        vp2[:, i],
                        x8_r[:, :, jt * P : (jt + 1) * P],
                        wmv,
                        start=True, stop=True, perf_mode=DR,
                    )
                eng_copy(copy_eng, vT[:, 2 * u : 2 * u + 2, :], vp2)

            # k1/k3 casts ride ScalarE (idle until the exp train starts);
            # the rest go to DVE
            kcopy_engs = [nc.vector, nc.scalar, nc.vector, nc.scalar,
                          nc.vector, nc.vector, nc.vector, nc.vector]
            for ck in range(8):
                kconv(ck, kcopy_engs[ck])

            xb = pe_.tile([P, CT, NQ], dt.bfloat16, tag="xb")

            # ---------- attention, per 512-wide query chunk ----------
            NIC = NQ // 512
            NU = 16  # jt pairs per chunk
            pend = {}

            def fin_a(ic):
                isl, a_ps, z_ps = pend[ic]
                acp = tmp.tile([P, CT, 512], dt.float32, tag="acp", name=f"acp{ic}")
                for ch in range(CT):
                    nc.vector.tensor_copy(acp[:, ch], a_ps[ch])
                zc = tmp.tile([1, 512], dt.float32, tag="zc", name=f"zc{ic}")
                nc.vector.tensor_scalar_mul(zc, z_ps, WSCALE)
                zb = tmp.tile([P, 2, 512], dt.float32, tag="zb", name=f"zb{ic}")
                nc.gpsimd.partition_broadcast(zb[:, 0], zc)
                nc.vector.reciprocal_approx_fast(zb[:, 1], zb[:, 0])
                pend[ic] = (isl, acp, zb[:, 1])

            def fin_b(ic):
                isl, acp, zr = pend.pop(ic)
                o_sb = tmp.tile([P, CT, 512], dt.float32, tag="o", name=f"o{ic}")
                for h in range(CT):
                    nc.vector.tensor_mul(o_sb[:, h], acp[:, h], zr)
                    nc.vector.tensor_add(o_sb[:, h], o_sb[:, h], xb[:, h, isl])
                    nc.sync.dma_start(out_ap[:, h, isl], o_sb[:, h])

            def fin_final(ic):
                # exposed tail: broadcast Z with a K=1 matmul (PE is free),
                # then stream the output in 256-wide pieces
                isl, a_ps, z_ps = pend.pop(ic)
                zc = tmp.tile([1, 512], dt.bfloat16, tag="zcf")
                nc.vector.tensor_scalar_mul(zc, z_ps, WSCALE)
                zb_ps = vpp.tile([P, 512], dt.float32, tag="vp")
                nc.tensor.matmul(zb_ps, ones_row, zc, start=True, stop=True)
                zr = tmp.tile([P, 512], dt.float32, tag="zrf")
                nc.vector.reciprocal_approx_fast(zr, zb_ps)
                o_sb = tmp.tile([P, CT, 512], dt.float32, tag="o", name="ofin")
                dma_engs = [nc.sync, nc.scalar, nc.sync, nc.scalar]
                for q in range(4):
                    h, hq = q // 2, q % 2
                    qs = slice(hq * 256, (hq + 1) * 256)
                    gsl = slice(isl.start + hq * 256, isl.start + (hq + 1) * 256)
                    oq = o_sb[:, h, qs]
                    # DVE does the PSUM-side muls; gpsimd (SBUF-only) chases
                    # with the residual adds so the two pipelines overlap
                    nc.vector.tensor_mul(oq, a_ps[h][:, qs], zr[:, qs])
                    nc.gpsimd.tensor_add(oq, oq, xb[:, h, gsl])
                    dma_engs[q].dma_start(out_ap[:, h, gsl], oq)

            # flat pair pipeline across chunk boundaries: the score/exp
            # lookahead never drains, so ScalarE keeps a 2-pair backlog
            # through every chunk transition
            pairs = [(ic, u) for ic in range(NIC) for u in range(NU)]
            isl_of = lambda ic: slice(ic * 512, (ic + 1) * 512)
            a_ps_of = {}
            z_ps_of = {}
            pts = {}

            def st_exp(ic, u):
                st2 = mmp.tile([P, 2, 512], dt.float32, tag="mm")
                for i in range(2):
                    jt = 2 * u + i
                    nc.tensor.matmul(
                        st2[:, i],
                        k_sb[:, :, jt * P : (jt + 1) * P],
                        x8_r[:, :, isl_of(ic)],
                        start=True, stop=True, perf_mode=DR,
                    )
                pt2 = ptp.tile([P, 2, 512], dt.float8e5, tag="pt")
                nc.scalar.activation(
                    pt2.rearrange("p a b -> p (a b)"),
                    st2.rearrange("p a b -> p (a b)"),
                    AF.Exp, scale=EXP_SCALE, bias=ebias,
                )
                pts[(ic, u)] = pt2

            vconv(0, nc.vector)
            vconv(1, nc.vector)
            st_exp(0, 0)
            st_exp(0, 1)
            for idx, (ic, u) in enumerate(pairs):
                if ic == 0 and u + 2 < NU:
                    vconv(u + 2, nc.vector)
                if idx + 2 < len(pairs):
                    st_exp(*pairs[idx + 2])
                if u == 0:
                    a_ps_of[ic] = [
                        accp.tile([P, 512], dt.float32, tag="acc", name=f"acc{ic}_{i}")
                        for i in range(CT)
                    ]
                    z_ps_of[ic] = zpp.tile([1, 512], dt.float32, tag="z", name=f"z{ic}")
                a_ps, z_ps = a_ps_of[ic], z_ps_of[ic]
                pt2 = pts.pop((ic, u))
                # last pair: Z first so the epilogue's Z chain starts under
                # the final PV matmuls
                if u == NU - 1:
                    nc.tensor.matmul(
                        z_ps, ones2, pt2, start=False, stop=True,
                        perf_mode=DR,
                    )
                for ch in range(CT):
                    nc.tensor.matmul(
                        a_ps[ch],
                        vT[:, 2 * u : 2 * u + 2, ch * P : (ch + 1) * P],
                        pt2,
                        start=(u == 0), stop=(u == NU - 1),
                        perf_mode=DR,
                    )
                if u < NU - 1:
                    nc.tensor.matmul(
                        z_ps, ones2, pt2,
                        start=(u == 0), stop=False,
                        perf_mode=DR,
                    )
                if ic == 0 and u == NU - 1:
                    # xb = x + proj-bias (bf16); lands in chunk 1's DVE lull
                    for h in range(CT):
                        for hf in range(2):
                            hs = slice(hf * 1024, (hf + 1) * 1024)
                            nc.vector.tensor_scalar_add(
                                xb[:, h, hs], xq_r[:, h, hs], bpp[:, h : h + 1]
                            )
                if u == NU - 1:
                    pend[ic] = (isl_of(ic), a_ps, z_ps)
                    if ic < NIC - 1:
                        fin_a(ic)
                    if ic > 0:
                        fin_b(ic - 1)
            fin_final(NIC - 1)

    nc.compile()
    return nc


def _get_nc():
    if "nc" not in _CACHED:
        _CACHED["nc"] = _build()
    return _CACHED["nc"]


def kernel(x, gn_scale, gn_bias, wq, bq, wk, bk, wv, bv, wp, bp, _trace=False, _trace_cores=None):
    try:
        import jax
        if jax.config.jax_compilation_cache_dir is None:
            jax.config.update("jax_compilation_cache_dir", "/tmp/attnblock_jax_cache")
            jax.config.update("jax_persistent_cache_min_compile_time_secs", 1.0)
    except Exception:
        pass
    import ml_dtypes
    from concourse.bass_utils import run_bass_kernel_spmd

    bf16 = ml_dtypes.bfloat16
    e4 = ml_dtypes.float8_e4m3
    nc = _get_nc()
    x = np.asarray(x, np.float32).reshape(B, C, N)

    def to_e4(a):
        return np.clip(a, -224.0, 224.0).astype(e4)

    def pack_w(w):
        # [c_out, c_in] -> lhsT layout [p, t*C + o] with c_in = t*128 + p
        wt = np.asarray(w, np.float64).T
        return np.ascontiguousarray(np.concatenate([wt[:P], wt[P:]], axis=1))

    wq64 = np.asarray(wq, np.float64)
    wk64 = np.asarray(wk, np.float64)
    wv64 = np.asarray(wv, np.float64)
    wp64 = np.asarray(wp, np.float64)
    mmat = wq64.T @ wk64
    wpv = wv64.T @ wp64.T
    gsc = np.asarray(gn_scale, np.float64)
    gbi = np.asarray(gn_bias, np.float64)
    bv64 = np.asarray(bv, np.float64)
    bp64 = np.asarray(bp, np.float64)

    # per-batch GroupNorm stats -> folded weights
    cg = C // NUM_GROUPS
    wm_b, aux_b = [], []
    rng = np.random.default_rng(0)
    sq = rng.choice(N, 48, replace=False)
    for b in range(B):
        xb64 = x[b].astype(np.float64)
        xg = xb64.reshape(NUM_GROUPS, cg, N)
        mean = xg.mean(axis=(1, 2))
        var = xg.var(axis=(1, 2))
        rstd = 1.0 / np.sqrt(var + EPS)
        alpha = np.repeat(rstd, cg) * gsc
        beta = gbi - np.repeat(mean * rstd, cg) * gsc
        A = (alpha[:, None] * mmat) * alpha[None, :]
        Wsv = alpha[:, None] * wpv
        bpp = bp64 + wp64 @ (bv64 + wv64 @ beta)
        # sampled score max -> shift (e5m2 makes overflow essentially
        # impossible; shift just centers P's dynamic range)
        ks = A @ xb64[:, sq]
        smax = float((ks.T @ xb64).max()) / 16.0
        shift = max(3.0, smax + 1.0 - 7.0)
        wmA = pack_w(WSCALE * A)
        wmV = np.concatenate([(WSCALE * Wsv)[:P], (WSCALE * Wsv)[P:]], axis=1)
        wm_b.append(to_e4(np.concatenate([wmA, wmV], axis=1)))
        aux = np.zeros((P, 8), np.float32)
        aux[:, 0] = bpp[:P]
        aux[:, 1] = bpp[P:]
        aux[:, 2] = -shift
        aux_b.append(aux)

    in_maps = []
    for core in range(8):
        b, qh = core // 2, core % 2
        xl = x[b] if qh == 0 else np.concatenate(
            [x[b][:, NQ:], x[b][:, :NQ]], axis=1
        )
        # pack to [p, t*N + n] with channel = t*128 + p
        xp = np.ascontiguousarray(np.concatenate([xl[:P], xl[P:]], axis=1))
        in_maps.append({
            "x8": to_e4(xp),
            "xq": np.ascontiguousarray(
                np.concatenate([xl[:P, :NQ], xl[P:, :NQ]], axis=1)
            ).astype(bf16),
            "wm": wm_b[b],
            "aux": aux_b[b],
        })

    last_err = None
    for attempt in range(3):
        try:
            res = run_bass_kernel_spmd(
                nc, in_maps, core_ids=list(range(8)), trace=_trace,
                trace_cores=_trace_cores,
            )
            break
        except Exception as e:  # transient NRT device faults happen rarely
            last_err = e
            import time as _time

            _time.sleep(2.0 * (attempt + 1))
    else:
        raise last_err
    out = np.empty((B, C, N), np.float32)
    for core in range(8):
        b, qh = core // 2, core % 2
        out[b][:, qh * NQ : (qh + 1) * NQ] = res.results[core]["out"]
    if _trace:
        _CACHED["last_results"] = res
    return out.reshape(B, C, H, W)


# revision 25
# speedup vs baseline: 1.0046x; 1.0046x over previous
"""AttnBlock (GroupNorm + single-head spatial self-attention + residual) on
8 Trainium2 NeuronCores — fp8 DoubleRow edition.

Sharding: batch (4) x query-half (2) -> 8 independent shards, one per core.
The host rolls the flattened spatial axis by 2048 for odd cores so each
core's queries are the first 2048 columns of its local x; K/V see all 4096.

Host preprocessing (all per batch, standard norm/weight folding):
  - GroupNorm stats (mean/var over 32ch x 4096) -> alpha/beta; folded into
    the conv weights:  A = diag(alpha) (Wq^T Wk) diag(alpha)  (scores
    bilinear form, the M-trick: per-query affine cancels under softmax,
    per-key O(mean) term dropped),  Wsv = diag(alpha) Wv^T Wp^T (V conv
    with the output projection folded in), bpp = bp + Wp(bv + Wv beta).
  - Weights shipped as fp8e4 scaled x16 (into fp8's sweet spot); x shipped
    twice: fp8e4 (matmul operand) and bf16 queries (residual).

Device pipeline per core, all matmuls fp8 DoubleRow (contraction 256 in one
pass, 2 MACs/cell/cycle):
  1. kconv: k_sb[c, n] = fp8(16 A x)    (8 x 2 DR matmuls, DMA-paced)
  2. vconv: vT[n, c]  = fp8(16 Wsv^T x) (32 DR matmuls, x stationary)
  3. 4 query chunks of 512: scores st[j,q] = k_sb^T x8 (DR, pair tiles in
     2 PSUM banks), P = exp(st/256 - shift) -> fp8e5 pair tiles (ScalarE,
     per-partition bias carries the shift; e5m2 makes overflow impossible),
     PV: a[c,q] += vT pair^T P pair (DR), Z accumulated on the PE with a
     [128,2,1] ones DoubleRow matmul per pair — no partition-reduction on
     DVE at all.
  4. Epilogue per chunk (DVE+GpSimd): a * 1/(16Z) + (x + bpp), streamed out.
"""
import numpy as np

B, C, H, W = 4, 256, 64, 64
N = H * W            # 4096 spatial positions
NQ = N // 2          # 2048 queries per core
P = 128              # partitions
CT = C // P          # 2 channel tiles
NUM_GROUPS = 8
EPS = 1e-5
WSCALE = 16.0        # fp8 weight prescale
EXP_SCALE = 1.0 / 256.0   # score descale: 1/16 (attn) * 1/16 (WSCALE)
WARM_MMS = 3

_CACHED = {}


def _build():
    import concourse.bass as bass
    import concourse.mybir as mybir
    import concourse.tile as tile
    from concourse import bacc

    dt = mybir.dt
    AF = mybir.ActivationFunctionType
    DR = mybir.MatmulPerfMode.DoubleRow

    nc = bacc.Bacc("TRN2", debug=False, num_devices=8)

    x8_d = nc.dram_tensor("x8", [P, CT * N], dt.float8e4, kind="ExternalInput")
    xq_d = nc.dram_tensor("xq", [P, CT * NQ], dt.bfloat16, kind="ExternalInput")
    # wm = [packed 16*A | packed 16*Wsv], each [P, CT*C]
    wm_d = nc.dram_tensor("wm", [P, 2 * CT * C], dt.float8e4, kind="ExternalInput")
    aux_d = nc.dram_tensor("aux", [P, 8], dt.float32, kind="ExternalInput")
    out_d = nc.dram_tensor("out", [C, NQ], dt.float32, kind="ExternalOutput")

    x8_ap = x8_d.ap()
    xq_ap = xq_d.ap()
    out_ap = out_d.ap().rearrange("(t p) n -> p t n", p=P)

    with tile.TileContext(nc) as tc:
        with (
            nc.allow_low_precision(reason="fp8 attention is intentional"),
            tc.tile_pool(name="persist", bufs=1) as pe_,
            tc.tile_pool(name="pt", bufs=6) as ptp,
            tc.tile_pool(name="tmp", bufs=3) as tmp,
            tc.tile_pool(name="mm", bufs=2, space="PSUM") as mmp,
            tc.tile_pool(name="acc", bufs=2, space="PSUM") as accp,
            tc.tile_pool(name="zp", bufs=1, space="PSUM") as zpp,
            tc.tile_pool(name="vp", bufs=1, space="PSUM") as vpp,
        ):
            # ---------- DMAs first: queue engines must trigger before any
            # other work lands on them (first transfer has ~3.5us ramp) ----
            x8_r = pe_.tile([P, CT, N], dt.float8e4, tag="x8")
            x8_flat = x8_r.rearrange("p t n -> p (t n)")
            wm_sb = pe_.tile([P, 2, CT, C], dt.float8e4, tag="wm")
            nc.gpsimd.dma_start(wm_sb.rearrange("p s t o -> p (s t o)"), wm_d.ap())
            # progressive chunk sizes: first kconv can start ~1.5us earlier
            edges = [0, 512, 1024, 2048, 3072, 4096]
            for ckb in range(len(edges) - 1):
                for t in range(CT):
                    fs = slice(t * N + edges[ckb], t * N + edges[ckb + 1])
                    eng = nc.sync if t == 0 else nc.scalar
                    eng.dma_start(x8_flat[:, fs], x8_ap[:, fs])
            aux_sb = pe_.tile([P, 8], dt.float32, tag="aux")
            nc.gpsimd.dma_start(aux_sb, aux_d.ap())
            xq_r = pe_.tile([P, CT, NQ], dt.bfloat16, tag="xq")
            nc.gpsimd.dma_start(
                xq_r.rearrange("p t n -> p (t n)"), xq_ap
            )

            # ---------- constants + PE warm-up ----------
            warm_w = pe_.tile([P, P], dt.bfloat16, tag="warmw")
            nc.vector.memset(warm_w, 0.0)
            warm_x = pe_.tile([P, 512], dt.bfloat16, tag="warmx")
            nc.vector.memset(warm_x, 0.0)
            # pair-dim byte stride must be %16 for DoubleRow ldweights
            ones2_t = pe_.tile([P, 2, 16], dt.float8e5, tag="ones2")
            nc.vector.memset(ones2_t.rearrange("p a b -> p (a b)"), 1.0)
            ones2 = ones2_t[:, :, 0:1]
            ones_row = pe_.tile([1, P], dt.bfloat16, tag="ones1r")
            nc.vector.memset(ones_row, 1.0)
            tjunk = pe_.tile([1, 2], dt.float32, tag="tjunk")
            nc.vector.memset(tjunk, 1.0)
            nc.scalar.activation(tjunk, tjunk, AF.Exp)
            for _ in range(WARM_MMS):
                wps = mmp.tile([P, 2, 512], dt.float32, tag="mm")
                nc.tensor.matmul(wps[:, 0], warm_w, warm_x, start=True, stop=True)
            wmk = wm_sb[:, 0]   # [P, CT, C] : 16*A packed
            wmv = wm_sb[:, 1]   # [P, CT, C] : 16*Wsv packed
            bpp = aux_sb[:, 0:2]
            ebias = aux_sb[:, 2:3]  # -shift

            # ---------- k conv (DoubleRow) ----------
            k_sb = pe_.tile([P, CT, N], dt.float8e4, tag="k")

            def eng_copy(eng, dst, src):
                if eng is nc.scalar:
                    eng.mul(dst, src, 1.0)
                else:
                    eng.tensor_copy(dst, src)

            def kconv(ck, copy_eng):
                s = slice(ck * 512, (ck + 1) * 512)
                cp2 = mmp.tile([P, 2, 512], dt.float32, tag="mm")
                for h in range(CT):
                    nc.tensor.matmul(
                        cp2[:, h],
                        wmk[:, :, h * P : (h + 1) * P],
                        x8_r[:, :, s],
                        start=True, stop=True, perf_mode=DR,
                    )
                eng_copy(copy_eng, k_sb[:, :, s], cp2)

            # ---------- v conv (DoubleRow, x stationary -> transposed out) ----------
            vT = pe_.tile([P, 32, C], dt.float8e4, tag="vT")

            def vconv(u, copy_eng):
                vp2 = vpp.tile([P, 2, C], dt.float32, tag="vp")
                for i in range(2):
                    jt = 2 * u + i
                    nc.tensor.matmul(
                        vp2[:, i],
                        x8_r[:, :, jt * P : (jt + 1) * P],
                        wmv,
                        start=True, stop=True, perf_mode=DR,
                    )
                eng_copy(copy_eng, vT[:, 2 * u : 2 * u + 2, :], vp2)

            # k1/k3 casts ride ScalarE (idle until the exp train starts);
            # the rest go to DVE
            kcopy_engs = [nc.vector, nc.scalar, nc.vector, nc.scalar,
                          nc.vector, nc.vector, nc.vector, nc.vector]
            for ck in range(8):
                kconv(ck, kcopy_engs[ck])

            xb = pe_.tile([P, CT, NQ], dt.bfloat16, tag="xb")

            # ---------- attention, per 512-wide query chunk ----------
            NIC = NQ // 512
            NU = 16  # jt pairs per chunk
            pend = {}

            def fin_a(ic):
                isl, a_ps, z_ps = pend[ic]
                acp = tmp.tile([P, CT, 512], dt.float32, tag="acp", name=f"acp{ic}")
                for ch in range(CT):
                    nc.vector.tensor_copy(acp[:, ch], a_ps[ch])
                zc = tmp.tile([1, 512], dt.float32, tag="zc", name=f"zc{ic}")
                nc.vector.tensor_scalar_mul(zc, z_ps, WSCALE)
                zb = tmp.tile([P, 2, 512], dt.float32, tag="zb", name=f"zb{ic}")
                nc.gpsimd.partition_broadcast(zb[:, 0], zc)
                nc.vector.reciprocal_approx_fast(zb[:, 1], zb[:, 0])
                pend[ic] = (isl, acp, zb[:, 1])

            def fin_b(ic):
                isl, acp, zr = pend.pop(ic)
                o_sb = tmp.tile([P, CT, 512], dt.float32, tag="o", name=f"o{ic}")
                for h in range(CT):
                    nc.vector.tensor_mul(o_sb[:, h], acp[:, h], zr)
                    nc.vector.tensor_add(o_sb[:, h], o_sb[:, h], xb[:, h, isl])
                    nc.sync.dma_start(out_ap[:, h, isl], o_sb[:, h])

            def fin_final(ic):
                # exposed tail: broadcast Z with a K=1 matmul (PE is free),
                # then stream the output in 256-wide pieces
                isl, a_ps, z_ps = pend.pop(ic)
                zc = tmp.tile([1, 512], dt.bfloat16, tag="zcf")
                nc.vector.tensor_scalar_mul(zc, z_ps, WSCALE)
                zb_ps = vpp.tile([P, 512], dt.float32, tag="vp")
                nc.tensor.matmul(zb_ps, ones_row, zc, start=True, stop=True)
                zr = tmp.tile([P, 512], dt.float32, tag="zrf")
                nc.vector.reciprocal_approx_fast(zr, zb_ps)
                o_sb = tmp.tile([P, CT, 512], dt.float32, tag="o", name="ofin")
                dma_engs = [nc.sync, nc.scalar, nc.sync, nc.scalar]
                for q in range(4):
                    h, hq = q // 2, q % 2
                    qs = slice(hq * 256, (hq + 1) * 256)
                    gsl = slice(isl.start + hq * 256, isl.start + (hq + 1) * 256)
                    oq = o_sb[:, h, qs]
                    # DVE does the PSUM-side muls; gpsimd (SBUF-only) chases
                    # with the residual adds so the two pipelines overlap
                    nc.vector.tensor_mul(oq, a_ps[h][:, qs], zr[:, qs])
                    nc.gpsimd.tensor_add(oq, oq, xb[:, h, gsl])
                    dma_engs[q].dma_start(out_ap[:, h, gsl], oq)

            # flat pair pipeline across chunk boundaries: the score/exp
            # lookahead never drains, so ScalarE keeps a 2-pair backlog
            # through every chunk transition
            pairs = [(ic, u) for ic in range(NIC) for u in range(NU)]
            isl_of = lambda ic: slice(ic * 512, (ic + 1) * 512)
            a_ps_of = {}
            z_ps_of = {}
            pts = {}

            def st_exp(ic, u):
                st2 = mmp.tile([P, 2, 512], dt.float32, tag="mm")
                for i in range(2):
                    jt = 2 * u + i
                    nc.tensor.matmul(
                        st2[:, i],
                        k_sb[:, :, jt * P : (jt + 1) * P],
                        x8_r[:, :, isl_of(ic)],
                        start=True, stop=True, perf_mode=DR,
                    )
                pt2 = ptp.tile([P, 2, 512], dt.float8e5, tag="pt")
                nc.scalar.activation(
                    pt2.rearrange("p a b -> p (a b)"),
                    st2.rearrange("p a b -> p (a b)"),
                    AF.Exp, scale=EXP_SCALE, bias=ebias,
                )
                pts[(ic, u)] = pt2

            vconv(0, nc.vector)
            vconv(1, nc.vector)
            st_exp(0, 0)
            st_exp(0, 1)
            for idx, (ic, u) in enumerate(pairs):
                if ic == 0 and u + 2 < NU:
                    vconv(u + 2, nc.vector)
                if idx + 2 < len(pairs):
                    st_exp(*pairs[idx + 2])
                if u == 0:
                    a_ps_of[ic] = [
                        accp.tile([P, 512], dt.float32, tag="acc", name=f"acc{ic}_{i}")
                        for i in range(CT)
                    ]
                    z_ps_of[ic] = zpp.tile([1, 512], dt.float32, tag="z", name=f"z{ic}")
                a_ps, z_ps = a_ps_of[ic], z_ps_of[ic]
                pt2 = pts.pop((ic, u))
                # last pair: Z first so the epilogue's Z chain starts under
                # the final PV matmuls
                if u == NU - 1:
                    nc.tensor.matmul(
                        z_ps, ones2, pt2, start=False, stop=True,
                        perf_mode=DR,
                    )
                for ch in range(CT):
                    nc.tensor.matmul(
                        a_ps[ch],
                        vT[:, 2 * u : 2 * u + 2, ch * P : (ch + 1) * P],
                        pt2,
                        start=(u == 0), stop=(u == NU - 1),
                        perf_mode=DR,
                    )
                if u < NU - 1:
                    nc.tensor.matmul(
                        z_ps, ones2, pt2,
                        start=(u == 0), stop=False,
                        perf_mode=DR,
                    )
                if ic == 0 and u == NU - 1:
                    # xb = x + proj-bias (bf16); lands in chunk 1's DVE lull
                    for h in range(CT):
                        for hf in range(2):
                            hs = slice(hf * 1024, (hf + 1) * 1024)
                            nc.vector.tensor_scalar_add(
                                xb[:, h, hs], xq_r[:, h, hs], bpp[:, h : h + 1]
                            )
                if u == NU - 1:
                    pend[ic] = (isl_of(ic), a_ps, z_ps)
                    if ic < NIC - 1:
                        fin_a(ic)
                    if ic > 0:
                        fin_b(ic - 1)
            fin_final(NIC - 1)

    nc.compile()
    return nc


def _get_nc():
    if "nc" not in _CACHED:
        _CACHED["nc"] = _build()
    return _CACHED["nc"]


def kernel(x, gn_scale, gn_bias, wq, bq, wk, bk, wv, bv, wp, bp, _trace=False, _trace_cores=None):
    try:
        import jax
        if jax.config.jax_compilation_cache_dir is None:
            jax.config.update("jax_compilation_cache_dir", "/tmp/attnblock_jax_cache")
            jax.config.update("jax_persistent_cache_min_compile_time_secs", 1.0)
    except Exception:
        pass
    import ml_dtypes
    from concourse.bass_utils import run_bass_kernel_spmd

    bf16 = ml_dtypes.bfloat16
    e4 = ml_dtypes.float8_e4m3
    nc = _get_nc()
    x = np.asarray(x, np.float32).reshape(B, C, N)

    def to_e4(a):
        return np.clip(a, -224.0, 224.0).astype(e4)

    def pack_w(w):
        # [c_out, c_in] -> lhsT layout [p, t*C + o] with c_in = t*128 + p
        wt = np.asarray(w, np.float64).T
        return np.ascontiguousarray(np.concatenate([wt[:P], wt[P:]], axis=1))

    wq64 = np.asarray(wq, np.float64)
    wk64 = np.asarray(wk, np.float64)
    wv64 = np.asarray(wv, np.float64)
    wp64 = np.asarray(wp, np.float64)
    mmat = wq64.T @ wk64
    wpv = wv64.T @ wp64.T
    gsc = np.asarray(gn_scale, np.float64)
    gbi = np.asarray(gn_bias, np.float64)
    bv64 = np.asarray(bv, np.float64)
    bp64 = np.asarray(bp, np.float64)

    # per-batch GroupNorm stats -> folded weights
    cg = C // NUM_GROUPS
    wm_b, aux_b = [], []
    rng = np.random.default_rng(0)
    sq = rng.choice(N, 48, replace=False)
    for b in range(B):
        xb64 = x[b].astype(np.float64)
        xg = xb64.reshape(NUM_GROUPS, cg, N)
        mean = xg.mean(axis=(1, 2))
        var = xg.var(axis=(1, 2))
        rstd = 1.0 / np.sqrt(var + EPS)
        alpha = np.repeat(rstd, cg) * gsc
        beta = gbi - np.repeat(mean * rstd, cg) * gsc
        A = (alpha[:, None] * mmat) * alpha[None, :]
        Wsv = alpha[:, None] * wpv
        bpp = bp64 + wp64 @ (bv64 + wv64 @ beta)
        # sampled score max -> shift (e5m2 makes overflow essentially
        # impossible; shift just centers P's dynamic range)
        ks = A @ xb64[:, sq]
        smax = float((ks.T @ xb64).max()) / 16.0
        shift = max(3.0, smax + 1.0 - 7.0)
        wmA = pack_w(WSCALE * A)
        wmV = np.concatenate([(WSCALE * Wsv)[:P], (WSCALE * Wsv)[P:]], axis=1)
        wm_b.append(to_e4(np.concatenate([wmA, wmV], axis=1)))
        aux = np.zeros((P, 8), np.float32)
        aux[:, 0] = bpp[:P]
        aux[:, 1] = bpp[P:]
        aux[:, 2] = -shift
        aux_b.append(aux)

    in_maps = []
    for core in range(8):
        b, qh = core // 2, core % 2
        xl = x[b] if qh == 0 else np.concatenate(
            [x[b][:, NQ:], x[b][:, :NQ]], axis=1
        )
        # pack to [p, t*N + n] with channel = t*128 + p
        xp = np.ascontiguousarray(np.concatenate([xl[:P], xl[P:]], axis=1))
        in_maps.append({
            "x8": to_e4(xp),
            "xq": np.ascontiguousarray(
                np.concatenate([xl[:P, :NQ], xl[P:, :NQ]], axis=1)
            ).astype(bf16),
            "wm": wm_b[b],
            "aux": aux_b[b],
        })

    last_err = None
    for attempt in range(3):
        try:
            res = run_bass_kernel_spmd(
                nc, in_maps, core_ids=list(range(8)), trace=_trace,
                trace_cores=_trace_cores,
            )
            break
        except Exception as e:  # transient NRT device faults happen rarely
            last_err = e
            import time as _time

            _time.sleep(2.0 * (attempt + 1))
    else:
        raise last_err
    out = np.empty((B, C, N), np.float32)
    for core in range(8):
        b, qh = core // 2, core % 2
        out[b][:, qh * NQ : (qh + 1) * NQ] = res.results[core]["out"]
    if _trace:
        _CACHED["last_results"] = res
    return out.reshape(B, C, H, W)


# revision 27
# speedup vs baseline: 1.0403x; 1.0356x over previous
"""AttnBlock (GroupNorm + single-head spatial self-attention + residual) on
8 Trainium2 NeuronCores — fp8 DoubleRow edition.

Sharding: batch (4) x query-half (2) -> 8 independent shards, one per core.
The host rolls the flattened spatial axis by 2048 for odd cores so each
core's queries are the first 2048 columns of its local x; K/V see all 4096.

Host preprocessing (all per batch, standard norm/weight folding):
  - GroupNorm stats (mean/var over 32ch x 4096) -> alpha/beta; folded into
    the conv weights:  A = diag(alpha) (Wq^T Wk) diag(alpha)  (scores
    bilinear form, the M-trick: per-query affine cancels under softmax,
    per-key O(mean) term dropped),  Wsv = diag(alpha) Wv^T Wp^T (V conv
    with the output projection folded in), bpp = bp + Wp(bv + Wv beta).
  - Weights shipped as fp8e4 scaled x16 (into fp8's sweet spot); x shipped
    twice: fp8e4 (matmul operand) and bf16 queries (residual).

Device pipeline per core, all matmuls fp8 DoubleRow (contraction 256 in one
pass, 2 MACs/cell/cycle):
  1. kconv: k_sb[c, n] = fp8(16 A x)    (8 x 2 DR matmuls, DMA-paced)
  2. vconv: vT[n, c]  = fp8(16 Wsv^T x) (32 DR matmuls, x stationary)
  3. 4 query chunks of 512: scores st[j,q] = k_sb^T x8 (DR, pair tiles in
     2 PSUM banks), P = exp(st/256 - shift) -> fp8e5 pair tiles (ScalarE,
     per-partition bias carries the shift; e5m2 makes overflow impossible),
     PV: a[c,q] += vT pair^T P pair (DR), Z accumulated on the PE with a
     [128,2,1] ones DoubleRow matmul per pair — no partition-reduction on
     DVE at all.
  4. Epilogue per chunk (DVE+GpSimd): a * 1/(16Z) + (x + bpp), streamed out.
"""
import numpy as np

B, C, H, W = 4, 256, 64, 64
N = H * W            # 4096 spatial positions
NQ = N // 2          # 2048 queries per core
P = 128              # partitions
CT = C // P          # 2 channel tiles
NUM_GROUPS = 8
EPS = 1e-5
WSCALE = 16.0        # fp8 weight prescale
EXP_SCALE = 1.0 / 256.0   # score descale: 1/16 (attn) * 1/16 (WSCALE)
WARM_MMS = 3

_CACHED = {}


def _build():
    import concourse.bass as bass
    import concourse.mybir as mybir
    import concourse.tile as tile
    from concourse import bacc

    dt = mybir.dt
    AF = mybir.ActivationFunctionType
    DR = mybir.MatmulPerfMode.DoubleRow

    nc = bacc.Bacc("TRN2", debug=False, num_devices=8)

    x8_d = nc.dram_tensor("x8", [P, CT * N], dt.float8e4, kind="ExternalInput")
    xq_d = nc.dram_tensor("xq", [P, CT * NQ], dt.bfloat16, kind="ExternalInput")
    # wm = [packed 16*A | packed 16*Wsv], each [P, CT*C]
    wm_d = nc.dram_tensor("wm", [P, 2 * CT * C], dt.float8e4, kind="ExternalInput")
    aux_d = nc.dram_tensor("aux", [P, 8], dt.float32, kind="ExternalInput")
    out_d = nc.dram_tensor("out", [C, NQ], dt.float32, kind="ExternalOutput")

    x8_ap = x8_d.ap()
    xq_ap = xq_d.ap()
    out_ap = out_d.ap().rearrange("(t p) n -> p t n", p=P)

    with tile.TileContext(nc) as tc:
        with (
            nc.allow_low_precision(reason="fp8 attention is intentional"),
            tc.tile_pool(name="persist", bufs=1) as pe_,
            tc.tile_pool(name="pt", bufs=6) as ptp,
            tc.tile_pool(name="tmp", bufs=3) as tmp,
            tc.tile_pool(name="mm", bufs=2, space="PSUM") as mmp,
            tc.tile_pool(name="acc", bufs=2, space="PSUM") as accp,
            tc.tile_pool(name="zp", bufs=1, space="PSUM") as zpp,
            tc.tile_pool(name="vp", bufs=1, space="PSUM") as vpp,
        ):
            # ---------- DMAs first: queue engines must trigger before any
            # other work lands on them (first transfer has ~3.5us ramp) ----
            x8_r = pe_.tile([P, CT, N], dt.float8e4, tag="x8")
            x8_flat = x8_r.rearrange("p t n -> p (t n)")
            wm_sb = pe_.tile([P, 2, CT, C], dt.float8e4, tag="wm")
            nc.gpsimd.dma_start(wm_sb.rearrange("p s t o -> p (s t o)"), wm_d.ap())
            edges = [0, 1024, 2048, 3072, 4096]
            for ckb in range(len(edges) - 1):
                for t in range(CT):
                    fs = slice(t * N + edges[ckb], t * N + edges[ckb + 1])
                    eng = nc.sync if t == 0 else nc.scalar
                    eng.dma_start(x8_flat[:, fs], x8_ap[:, fs])
            aux_sb = pe_.tile([P, 8], dt.float32, tag="aux")
            nc.gpsimd.dma_start(aux_sb, aux_d.ap())
            xq_r = pe_.tile([P, CT, NQ], dt.bfloat16, tag="xq")
            nc.gpsimd.dma_start(
                xq_r.rearrange("p t n -> p (t n)"), xq_ap
            )

            # ---------- constants + PE warm-up ----------
            warm_w = pe_.tile([P, P], dt.bfloat16, tag="warmw")
            nc.vector.memset(warm_w, 0.0)
            warm_x = pe_.tile([P, 512], dt.bfloat16, tag="warmx")
            nc.vector.memset(warm_x, 0.0)
            # pair-dim byte stride must be %16 for DoubleRow ldweights
            ones2_t = pe_.tile([P, 2, 16], dt.float8e5, tag="ones2")
            nc.vector.memset(ones2_t.rearrange("p a b -> p (a b)"), 1.0)
            ones2 = ones2_t[:, :, 0:1]
            ones_row = pe_.tile([1, P], dt.bfloat16, tag="ones1r")
            nc.vector.memset(ones_row, 1.0)
            tjunk = pe_.tile([1, 2], dt.float32, tag="tjunk")
            nc.vector.memset(tjunk, 1.0)
            nc.scalar.activation(tjunk, tjunk, AF.Exp)
            for _ in range(WARM_MMS):
                wps = mmp.tile([P, 2, 512], dt.float32, tag="mm")
                nc.tensor.matmul(wps[:, 0], warm_w, warm_x, start=True, stop=True)
            wmk = wm_sb[:, 0]   # [P, CT, C] : 16*A packed
            wmv = wm_sb[:, 1]   # [P, CT, C] : 16*Wsv packed
            bpp = aux_sb[:, 0:2]
            ebias = aux_sb[:, 2:3]  # -shift

            # ---------- k conv (DoubleRow) ----------
            k_sb = pe_.tile([P, CT, N], dt.float8e4, tag="k")

            def eng_copy(eng, dst, src):
                if eng is nc.scalar:
                    eng.mul(dst, src, 1.0)
                else:
                    eng.tensor_copy(dst, src)

            def kconv(ck, copy_eng):
                s = slice(ck * 512, (ck + 1) * 512)
                cp2 = mmp.tile([P, 2, 512], dt.float32, tag="mm")
                for h in range(CT):
                    nc.tensor.matmul(
                        cp2[:, h],
                        wmk[:, :, h * P : (h + 1) * P],
                        x8_r[:, :, s],
                        start=True, stop=True, perf_mode=DR,
                    )
                eng_copy(copy_eng, k_sb[:, :, s], cp2)

            # ---------- v conv (DoubleRow, x stationary -> transposed out) ----------
            vT = pe_.tile([P, 32, C], dt.float8e4, tag="vT")

            def vconv(u, copy_eng):
                vp2 = vpp.tile([P, 2, C], dt.float32, tag="vp")
                for i in range(2):
                    jt = 2 * u + i
                    nc.tensor.matmul(
                        vp2[:, i],
                        x8_r[:, :, jt * P : (jt + 1) * P],
                        wmv,
                        start=True, stop=True, perf_mode=DR,
                    )
                eng_copy(copy_eng, vT[:, 2 * u : 2 * u + 2, :], vp2)

            kcopy_engs = [nc.vector, nc.scalar] * 4
            for ck in range(8):
                kconv(ck, kcopy_engs[ck])

            xb = pe_.tile([P, CT, NQ], dt.bfloat16, tag="xb")

            # ---------- attention, per 512-wide query chunk ----------
            NIC = NQ // 512
            NU = 16  # jt pairs per chunk
            pend = {}

            def fin_a(ic):
                isl, a_ps, z_ps = pend[ic]
                acp = tmp.tile([P, CT, 512], dt.float32, tag="acp", name=f"acp{ic}")
                for ch in range(CT):
                    nc.vector.tensor_copy(acp[:, ch], a_ps[ch])
                zc = tmp.tile([1, 512], dt.float32, tag="zc", name=f"zc{ic}")
                nc.vector.tensor_scalar_mul(zc, z_ps, WSCALE)
                zb = tmp.tile([P, 2, 512], dt.float32, tag="zb", name=f"zb{ic}")
                nc.gpsimd.partition_broadcast(zb[:, 0], zc)
                nc.vector.reciprocal_approx_fast(zb[:, 1], zb[:, 0])
                pend[ic] = (isl, acp, zb[:, 1])

            def fin_b(ic):
                isl, acp, zr = pend.pop(ic)
                o_sb = tmp.tile([P, CT, 512], dt.float32, tag="o", name=f"o{ic}")
                for h in range(CT):
                    nc.vector.tensor_mul(o_sb[:, h], acp[:, h], zr)
                    nc.vector.tensor_add(o_sb[:, h], o_sb[:, h], xb[:, h, isl])
                    nc.sync.dma_start(out_ap[:, h, isl], o_sb[:, h])

            def fin_final(ic):
                # exposed tail: broadcast Z with a K=1 matmul (PE is free),
                # then stream the output in 256-wide pieces
                isl, a_ps, z_ps = pend.pop(ic)
                zc = tmp.tile([1, 512], dt.bfloat16, tag="zcf")
                nc.vector.tensor_scalar_mul(zc, z_ps, WSCALE)
                zb_ps = vpp.tile([P, 512], dt.float32, tag="vp")
                nc.tensor.matmul(zb_ps, ones_row, zc, start=True, stop=True)
                zr = tmp.tile([P, 512], dt.float32, tag="zrf")
                nc.vector.reciprocal_approx_fast(zr, zb_ps)
                o_sb = tmp.tile([P, CT, 512], dt.float32, tag="o", name="ofin")
                dma_engs = [nc.sync, nc.scalar, nc.sync, nc.scalar]
                for q in range(4):
                    h, hq = q // 2, q % 2
                    qs = slice(hq * 256, (hq + 1) * 256)
                    gsl = slice(isl.start + hq * 256, isl.start + (hq + 1) * 256)
                    oq = o_sb[:, h, qs]
                    # DVE does the PSUM-side muls; gpsimd (SBUF-only) chases
                    # with the residual adds so the two pipelines overlap
                    nc.vector.tensor_mul(oq, a_ps[h][:, qs], zr[:, qs])
                    nc.gpsimd.tensor_add(oq, oq, xb[:, h, gsl])
                    dma_engs[q].dma_start(out_ap[:, h, gsl], oq)

            # flat pair pipeline across chunk boundaries: the score/exp
            # lookahead never drains, so ScalarE keeps a 2-pair backlog
            # through every chunk transition
            pairs = [(ic, u) for ic in range(NIC) for u in range(NU)]
            isl_of = lambda ic: slice(ic * 512, (ic + 1) * 512)
            a_ps_of = {}
            z_ps_of = {}
            pts = {}

            def st_exp(ic, u):
                st2 = mmp.tile([P, 2, 512], dt.float32, tag="mm")
                for i in range(2):
                    jt = 2 * u + i
                    nc.tensor.matmul(
                        st2[:, i],
                        k_sb[:, :, jt * P : (jt + 1) * P],
                        x8_r[:, :, isl_of(ic)],
                        start=True, stop=True, perf_mode=DR,
                    )
                pt2 = ptp.tile([P, 2, 512], dt.float8e5, tag="pt")
                nc.scalar.activation(
                    pt2.rearrange("p a b -> p (a b)"),
                    st2.rearrange("p a b -> p (a b)"),
                    AF.Exp, scale=EXP_SCALE, bias=ebias,
                )
                pts[(ic, u)] = pt2

            vconv(0, nc.vector)
            vconv(1, nc.vector)
            st_exp(0, 0)
            st_exp(0, 1)
            for idx, (ic, u) in enumerate(pairs):
                if ic == 0 and u + 2 < NU:
                    vconv(u + 2, nc.vector)
                if idx + 2 < len(pairs):
                    st_exp(*pairs[idx + 2])
                if u == 0:
                    a_ps_of[ic] = [
                        accp.tile([P, 512], dt.float32, tag="acc", name=f"acc{ic}_{i}")
                        for i in range(CT)
                    ]
                    z_ps_of[ic] = zpp.tile([1, 512], dt.float32, tag="z", name=f"z{ic}")
                a_ps, z_ps = a_ps_of[ic], z_ps_of[ic]
                pt2 = pts.pop((ic, u))
                # last pair: Z first so the epilogue's Z chain starts under
                # the final PV matmuls
                if u == NU - 1:
                    nc.tensor.matmul(
                        z_ps, ones2, pt2, start=False, stop=True,
                        perf_mode=DR,
                    )
                for ch in range(CT):
                    nc.tensor.matmul(
                        a_ps[ch],
                        vT[:, 2 * u : 2 * u + 2, ch * P : (ch + 1) * P],
                        pt2,
                        start=(u == 0), stop=(u == NU - 1),
                        perf_mode=DR,
                    )
                if u < NU - 1:
                    nc.tensor.matmul(
                        z_ps, ones2, pt2,
                        start=(u == 0), stop=False,
                        perf_mode=DR,
                    )
                if ic == 0 and u == NU - 1:
                    # xb = x + proj-bias (bf16); lands in chunk 1's DVE lull
                    for h in range(CT):
                        for hf in range(2):
                            hs = slice(hf * 1024, (hf + 1) * 1024)
                            nc.vector.tensor_scalar_add(
                                xb[:, h, hs], xq_r[:, h, hs], bpp[:, h : h + 1]
                            )
                if u == NU - 1:
                    pend[ic] = (isl_of(ic), a_ps, z_ps)
                    if ic < NIC - 1:
                        fin_a(ic)
                    if ic > 0:
                        fin_b(ic - 1)
            fin_final(NIC - 1)

    nc.compile()
    return nc


def _get_nc():
    if "nc" not in _CACHED:
        _CACHED["nc"] = _build()
    return _CACHED["nc"]


def kernel(x, gn_scale, gn_bias, wq, bq, wk, bk, wv, bv, wp, bp, _trace=False, _trace_cores=None):
    try:
        import jax
        if jax.config.jax_compilation_cache_dir is None:
            jax.config.update("jax_compilation_cache_dir", "/tmp/attnblock_jax_cache")
            jax.config.update("jax_persistent_cache_min_compile_time_secs", 1.0)
    except Exception:
        pass
    import ml_dtypes
    from concourse.bass_utils import run_bass_kernel_spmd

    bf16 = ml_dtypes.bfloat16
    e4 = ml_dtypes.float8_e4m3
    nc = _get_nc()
    x = np.asarray(x, np.float32).reshape(B, C, N)

    def to_e4(a):
        return np.clip(a, -224.0, 224.0).astype(e4)

    def pack_w(w):
        # [c_out, c_in] -> lhsT layout [p, t*C + o] with c_in = t*128 + p
        wt = np.asarray(w, np.float64).T
        return np.ascontiguousarray(np.concatenate([wt[:P], wt[P:]], axis=1))

    wq64 = np.asarray(wq, np.float64)
    wk64 = np.asarray(wk, np.float64)
    wv64 = np.asarray(wv, np.float64)
    wp64 = np.asarray(wp, np.float64)
    mmat = wq64.T @ wk64
    wpv = wv64.T @ wp64.T
    gsc = np.asarray(gn_scale, np.float64)
    gbi = np.asarray(gn_bias, np.float64)
    bv64 = np.asarray(bv, np.float64)
    bp64 = np.asarray(bp, np.float64)

    # per-batch GroupNorm stats -> folded weights
    cg = C // NUM_GROUPS
    wm_b, aux_b = [], []
    rng = np.random.default_rng(0)
    sq = rng.choice(N, 48, replace=False)
    for b in range(B):
        xb64 = x[b].astype(np.float64)
        xg = xb64.reshape(NUM_GROUPS, cg, N)
        mean = xg.mean(axis=(1, 2))
        var = xg.var(axis=(1, 2))
        rstd = 1.0 / np.sqrt(var + EPS)
        alpha = np.repeat(rstd, cg) * gsc
        beta = gbi - np.repeat(mean * rstd, cg) * gsc
        A = (alpha[:, None] * mmat) * alpha[None, :]
        Wsv = alpha[:, None] * wpv
        bpp = bp64 + wp64 @ (bv64 + wv64 @ beta)
        # sampled score max -> shift (e5m2 makes overflow essentially
        # impossible; shift just centers P's dynamic range)
        ks = A @ xb64[:, sq]
        smax = float((ks.T @ xb64).max()) / 16.0
        shift = max(3.0, smax + 1.0 - 7.0)
        wmA = pack_w(WSCALE * A)
        wmV = np.concatenate([(WSCALE * Wsv)[:P], (WSCALE * Wsv)[P:]], axis=1)
        wm_b.append(to_e4(np.concatenate([wmA, wmV], axis=1)))
        aux = np.zeros((P, 8), np.float32)
        aux[:, 0] = bpp[:P]
        aux[:, 1] = bpp[P:]
        aux[:, 2] = -shift
        aux_b.append(aux)

    in_maps = []
    for core in range(8):
        b, qh = core // 2, core % 2
        xl = x[b] if qh == 0 else np.concatenate(
            [x[b][:, NQ:], x[b][:, :NQ]], axis=1
        )
        # pack to [p, t*N + n] with channel = t*128 + p
        xp = np.ascontiguousarray(np.concatenate([xl[:P], xl[P:]], axis=1))
        in_maps.append({
            "x8": to_e4(xp),
            "xq": np.ascontiguousarray(
                np.concatenate([xl[:P, :NQ], xl[P:, :NQ]], axis=1)
            ).astype(bf16),
            "wm": wm_b[b],
            "aux": aux_b[b],
        })

    last_err = None
    for attempt in range(3):
        try:
            res = run_bass_kernel_spmd(
                nc, in_maps, core_ids=list(range(8)), trace=_trace,
                trace_cores=_trace_cores,
            )
            break
        except Exception as e:  # transient NRT device faults happen rarely
            last_err = e
            import time as _time

            _time.sleep(2.0 * (attempt + 1))
    else:
        raise last_err
    out = np.empty((B, C, N), np.float32)
    for core in range(8):
        b, qh = core // 2, core % 2
        out[b][:, qh * NQ : (qh + 1) * NQ] = res.results[core]["out"]
    if _trace:
        _CACHED["last_results"] = res
    return out.reshape(B, C, H, W)
